# revision 24
# baseline (speedup 1.0000x reference)
"""GroupedQueryAttention TRN2 Bass kernel (v3).

Strategy (8 NeuronCores, tensor-parallel over heads):
  - Each core owns 4 q-heads (one kv head, GQA group of 4), all tokens.
  - Single fully-interleaved emission stream so the PE never idles (keeps
    the HAM clock gate at 2.4 GHz) and exp (ACT) overlaps matmuls:
      proj chunk 0..7 (QKV + RoPE)  interleaved with  pair-0 attention
      stripes as their token chunks complete; pair-1 attention interleaved
      with pair-0 out-projection chunks; tail = last 2 small A2A pieces +
      2 out-proj chunks.
  - Attention per (head-pair, batch, q-stripe, k-block): S = K^T Q for 2
    heads, exp on ACT, ctx^T += V_aug^T exp (ones column gives the softmax
    denominator), normalize via reciprocal + partition broadcast.
  - Re-shard head->token via 16 small AllToAlls: one per (pair, batch,
    token-half-of-256) = 256 KB each, fired as soon as that (pair, batch)
    finishes.  Core c owns tokens [c*256,(c+1)*256) of EACH batch.
  - Out-proj: two passes (pair-0 into bf16 partial, pair-1 adds), emitted
    per 128-token chunk right after its A2A piece; Wo streamed in two
    4 MB halves through one SBUF slot.
"""

import os
import sys

import numpy as np


def _ensure_concourse():
    try:
        import concourse.bass  # noqa: F401
    except ImportError:
        for p in ("/opt/trn_rl_repo", "/root/.axon_site/_ro/trn_rl_repo"):
            if os.path.isdir(p) and p not in sys.path:
                sys.path.insert(0, p)
        import concourse.bass  # noqa: F401


FULL_CFG = dict(B=2, S=2048, E=2048, NH=32, NKV=8, HD=64, ncores=8, IC=512)

LAST_RESULTS = None
_CACHED_NC = None


def build_gqa(cfg):
    """Build the Bass module for one core's SPMD program. Returns nc."""
    _ensure_concourse()
    from contextlib import ExitStack

    import concourse.mybir as mybir
    import concourse.tile as tile
    from concourse import bacc
    from concourse.masks import make_identity

    dt = mybir.dt
    f32 = dt.float32
    bf16 = dt.bfloat16
    Exp = mybir.ActivationFunctionType.Exp

    B, S, E = cfg["B"], cfg["S"], cfg["E"]
    NH, NKV, HD = cfg["NH"], cfg["NKV"], cfg["HD"]
    NCORES = cfg["ncores"]
    HPC = NH // NCORES          # 4 q heads per core
    assert HPC == 4 and HD == 64
    QH = HPC * HD               # 256 q rows per core
    KVD = 2 * HD                # 128 packed K|V projection width
    NI = B * S                  # 4096 tokens
    ET = E // 128               # 16 contraction tiles
    IC = cfg["IC"]              # phase-1 token chunk (512)
    QB = 512                    # attention q stripe
    KB = 128                    # attention k block
    NQT = S // QB               # 4 stripes per batch
    SKT = S // KB               # 16 k tiles per batch
    NKTILES = NI // KB          # 32 k tiles
    TPB = 256                   # output tokens per (core, batch)
    TOK = B * TPB               # 512 output tokens per core
    scale = 1.0 / float(np.sqrt(HD))

    nc = bacc.Bacc("TRN2", target_bir_lowering=False, debug=False,
                   num_devices=NCORES)

    xT = nc.dram_tensor("xT", [E, NI], bf16, kind="ExternalInput").ap()
    wqT = nc.dram_tensor("wqT", [E, QH], bf16, kind="ExternalInput").ap()
    wkvT = nc.dram_tensor("wkvT", [E, KVD], bf16, kind="ExternalInput").ap()
    # host pre-interleaved: rows [m*1024 + d*128 + :128] = Wo^T block for
    # (src core d, head-pair m)
    woT = nc.dram_tensor("woT", [E, E], bf16, kind="ExternalInput").ap()
    cosT = nc.dram_tensor("cosT", [128, S], bf16, kind="ExternalInput").ap()
    sinT = nc.dram_tensor("sinT", [128, S], bf16, kind="ExternalInput").ap()
    # rows [b*TPB + j*128 + :128] = batch b, own tokens [j*128:(j+1)*128)
    outT = nc.dram_tensor("outT", [TOK, E], bf16, kind="ExternalOutput").ap()

    with tile.TileContext(nc) as tc, ExitStack() as persist:
        const = persist.enter_context(tc.tile_pool(name="const", bufs=1))
        qt_pool = persist.enter_context(tc.tile_pool(name="qt", bufs=1))
        kt_pool = persist.enter_context(tc.tile_pool(name="kt", bufs=1))
        vaug_pool = persist.enter_context(tc.tile_pool(name="vaug", bufs=1))
        wo_pool = persist.enter_context(tc.tile_pool(name="wo", bufs=1))
        ct_pool = persist.enter_context(tc.tile_pool(name="ct", bufs=1))
        pp_pool = persist.enter_context(tc.tile_pool(name="pp", bufs=4))
        dram = persist.enter_context(
            tc.tile_pool(name="dram", bufs=1, space="DRAM"))

        ident = const.tile([128, 128], bf16, name="ident", tag="ident")
        make_identity(nc, ident[:, :])
        # startup DMAs spread across rings: wq + x chunks on sync; cos/sin
        # then wo-half-0 on scalar; wkv on vector.
        wq_sb = const.tile([128, ET, QH], bf16, name="wq_sb", tag="wq")
        for ts in range(0, ET, 4):
            nc.scalar.dma_start(
                wq_sb[:, ts:ts + 4, :],
                wqT[ts * 128:(ts + 4) * 128, :].rearrange(
                    "(t p) o -> p t o", p=128))
        wkv_sb = const.tile([128, ET, KVD], bf16, name="wkv_sb", tag="wkv")
        nc.gpsimd.dma_start(wkv_sb[:, :, :],
                            wkvT.rearrange("(t p) o -> p t o", p=128))
        cos_sb = const.tile([128, S], bf16, name="cos_sb", tag="cos")
        nc.scalar.dma_start(cos_sb[:, :], cosT)
        sin_sb = const.tile([128, S], bf16, name="sin_sb", tag="sin")
        nc.scalar.dma_start(sin_sb[:, :], sinT)

        def load_wo_half(m):
            wo_sb = wo_pool.tile([128, NCORES, E], bf16, name=f"wo{m}",
                                 tag="wo")
            for d in range(NCORES):
                r0 = (m * NCORES + d) * 128
                nc.scalar.dma_start(wo_sb[:, d, :], woT[r0:r0 + 128, :])
            return wo_sb

        wo_sb = [None, None]
        wo_sb[0] = load_wo_half(0)

        # triangular causal mask for the diagonal 128-block, dup for 2 heads
        tri = const.tile([128, 2, 128], bf16, name="tri", tag="tri")
        nc.gpsimd.memset(tri[:, :, :], 1.0)
        nc.gpsimd.affine_select(
            out=tri[:, :, :], in_=tri[:, :, :],
            pattern=[[0, 2], [1, 128]], compare_op=mybir.AluOpType.is_ge,
            fill=0.0, base=0, channel_multiplier=-1)

        # persistent activations
        qt_sb = qt_pool.tile([64, HPC, NI], bf16, name="qt", tag="qt")
        kt_sb = kt_pool.tile([64, NI], bf16, name="kt", tag="kt")
        vaug_all = vaug_pool.tile([128, NKTILES, 2 * HD], bf16,
                                  name="vaug", tag="vaug")
        nc.vector.memset(vaug_all[:, :, :], 0.0)
        nc.vector.memset(vaug_all[:, :, 0:1], 1.0)
        vaug = [vaug_all[:, k, :] for k in range(NKTILES)]

        # collective buffers: per (pair m, batch b, token-half j):
        # [dest core, 128 rows, 128 tokens] = 256 KB
        cc_in = [[[dram.tile([NCORES, 128, 128], bf16,
                             name=f"cci{m}{b}{j}", tag=f"cci{m}{b}{j}")
                   for j in range(2)] for b in range(B)] for m in range(2)]
        cc_out = [[[dram.tile([NCORES, 128, 128], bf16,
                              name=f"cco{m}{b}{j}", tag=f"cco{m}{b}{j}")
                    for j in range(2)] for b in range(B)] for m in range(2)]
        # re-sharded ctx: [128 rows, src core, 128 tokens] per (m, b, j)
        ct = [[[ct_pool.tile([128, NCORES, 128], bf16,
                             name=f"ct{m}{b}{j}", tag=f"ct{m}{b}{j}")
                for j in range(2)] for b in range(B)] for m in range(2)]

        # ---------------- pools for the interleaved stream ----------------
        xt_pool = persist.enter_context(tc.tile_pool(name="xt", bufs=2))
        rope_pool = persist.enter_context(tc.tile_pool(name="rope", bufs=2))
        vs_pool = persist.enter_context(tc.tile_pool(name="vs", bufs=1))
        scores_ps = persist.enter_context(
            tc.tile_pool(name="scores_ps", bufs=2, space="PSUM"))
        ctx_ps_pool = persist.enter_context(
            tc.tile_pool(name="ctx_ps", bufs=1, space="PSUM"))
        et_pool = persist.enter_context(tc.tile_pool(name="et", bufs=3))
        rc_pool = persist.enter_context(tc.tile_pool(name="rc", bufs=1))
        cx_pool = persist.enter_context(tc.tile_pool(name="cx", bufs=1))
        rb_pool = persist.enter_context(tc.tile_pool(name="rb", bufs=1))
        st_pool = persist.enter_context(tc.tile_pool(name="st", bufs=2))
        ob_pool = persist.enter_context(tc.tile_pool(name="ob", bufs=2))
        # proj psum (QKV projections + V transposes) is allocated LAST so it
        # can be released (stack order) after the final chunk, freeing its
        # banks for the out-projection psum pool.
        from contextlib import ExitStack as _ES
        proj_stack = _ES()
        proj_ps_pool = proj_stack.enter_context(
            tc.tile_pool(name="proj_ps", bufs=2, space="PSUM"))
        pass_ps = [None]

        # persistent band tiles with pre-zeroed garbage region so band
        # exps write only [q0:] and ctx can read the full tile
        e_band = [et_pool.tile([128, 2, QB], bf16, name=f"eb{j}",
                               tag=f"eb{j}", bufs=1)
                  for j in range(QB // KB)]
        for j in range(1, QB // KB):
            nc.gpsimd.memset(e_band[j][:, :, 0:j * KB], 0.0)

        # ---------------- emit helpers ----------------
        def rope(src_ps, parts, s0, dsts):
            # rotate-half via partition-shifted PSUM reads (PSUM operands
            # are exempt from the SBUF same-start-partition rule), so no
            # swap copies are needed and ACT stays exp-only.
            t1 = rope_pool.tile([128, IC], bf16, name="t1", tag="t1")
            sw = rope_pool.tile([128, IC], bf16, name="sw", tag="sw")
            nc.vector.tensor_mul(t1[:parts, :], src_ps[:parts, :],
                                 cos_sb[:parts, s0:s0 + IC])
            for h0 in range(0, parts, 64):
                nc.vector.tensor_mul(sw[h0:h0 + 32, :],
                                     src_ps[h0 + 32:h0 + 64, :],
                                     sin_sb[h0:h0 + 32, s0:s0 + IC])
                nc.vector.tensor_mul(sw[h0 + 32:h0 + 64, :],
                                     src_ps[h0:h0 + 32, :],
                                     sin_sb[h0 + 32:h0 + 64, s0:s0 + IC])
            for out_ap, r0 in dsts:
                nc.vector.tensor_add(out_ap, t1[r0:r0 + 64, :],
                                     sw[r0:r0 + 64, :])

        # pending attention jobs, drained one-at-a-time between matmul
        # sub-chains so the ACT exp stream never starves while the PE works
        # through projection / out-projection chains (and vice versa)
        job_queue = []

        def feed(n=1):
            for _ in range(min(n, len(job_queue))):
                emit_job(job_queue.pop(0))

        def emit_chunk(ch):
            i0 = ch * IC
            s0 = i0 % S
            xt = xt_pool.tile([128, ET, IC], bf16, name="xt", tag="xt")
            for ts in range(0, ET, 4):
                nc.sync.dma_start(
                    xt[:, ts:ts + 4, :],
                    xT[ts * 128:(ts + 4) * 128, i0:i0 + IC].rearrange(
                        "(t p) i -> p t i", p=128))
            for m in range(2):
                q_ps = proj_ps_pool.tile([128, IC], f32, name="pps",
                                         tag="proj")
                for t in range(ET):
                    nc.tensor.matmul(
                        q_ps[:, :],
                        wq_sb[:, t, m * 128:(m + 1) * 128],
                        xt[:, t, :],
                        start=(t == 0), stop=(t == ET - 1))
                    if t % 4 == 3:
                        feed()
                rope(q_ps, 128, s0,
                     [(qt_sb[0:64, 2 * m, i0:i0 + IC], 0),
                      (qt_sb[0:64, 2 * m + 1, i0:i0 + IC], 64)])
                feed()
            kv_ps = proj_ps_pool.tile([128, IC], f32, name="pps", tag="proj")
            for t in range(ET):
                nc.tensor.matmul(
                    kv_ps[:, :],
                    wkv_sb[:, t, :],
                    xt[:, t, :],
                    start=(t == 0), stop=(t == ET - 1))
                if t % 4 == 3:
                    feed()
            rope(kv_ps, 64, s0, [(kt_sb[0:64, i0:i0 + IC], 0)])
            vs = vs_pool.tile([64, IC], bf16, name="vs", tag="vs")
            nc.vector.tensor_copy(vs[:, :], kv_ps[64:128, :])
            feed()
            for j in range(IC // 128):
                kidx = (i0 + j * 128) // 128
                vt_ps = proj_ps_pool.tile([128, HD], bf16, name="vt",
                                          tag="proj")
                nc.tensor.transpose(vt_ps[:, :],
                                    vs[:, j * 128:(j + 1) * 128],
                                    ident[0:64, 0:64])
                nc.vector.tensor_copy(vaug[kidx][:, HD:2 * HD], vt_ps[:, :])
                feed()

        # ---------------- attention ----------------
        def emit_scores(job):
            m, b, qt, kt, nkt = job
            sl = b * S + qt * QB
            j = kt - qt * (QB // KB)
            kp = b * S + kt * KB
            s_ps = scores_ps.tile([128, 2, QB], f32, name="sps", tag="sps")
            if j < 0:
                e_t = et_pool.tile([128, 2, QB], bf16, name="et", tag="et")
                for h in range(2):
                    nc.tensor.matmul(
                        s_ps[:, h, :],
                        kt_sb[0:64, kp:kp + KB],
                        qt_sb[0:64, 2 * m + h, sl:sl + QB],
                        start=True, stop=True)
                nc.scalar.activation(e_t[:, :, :], s_ps[:, :, :], Exp,
                                     scale=scale)
            else:
                e_t = e_band[j]
                q0 = j * KB
                for h in range(2):
                    nc.tensor.matmul(
                        s_ps[:, h, q0:QB],
                        kt_sb[0:64, kp:kp + KB],
                        qt_sb[0:64, 2 * m + h, sl + q0:sl + QB],
                        start=True, stop=True)
                nc.scalar.activation(e_t[:, :, q0:QB],
                                     s_ps[:, :, q0:QB], Exp,
                                     scale=scale)
                nc.vector.tensor_mul(e_t[:, :, q0:q0 + KB],
                                     e_t[:, :, q0:q0 + KB],
                                     tri[:, :, :])
            return e_t

        ctx_cur = [None]

        def emit_ctx(job, e_t):
            m, b, qt, kt, nkt = job
            if kt == 0:
                ctx_cur[0] = ctx_ps_pool.tile([128, 2, QB], f32,
                                              name="ctx", tag="ctx")
            ctx_ps = ctx_cur[0]
            j = kt - qt * (QB // KB)
            q0 = max(j, 0) * KB  # zero region of band tiles: skip it
            for h in range(2):
                nc.tensor.matmul(
                    ctx_ps[:, h, q0:QB],
                    vaug[b * SKT + kt][:, :],
                    e_t[:, h, q0:QB],
                    start=(kt == 0), stop=(kt == nkt - 1),
                    skip_group_check=(q0 > 0))
            if kt != nkt - 1:
                return
            # decouple: two fast copies free the ctx psum bank for the next
            # stripe; the normalize chain (recip -> broadcast -> mul) then
            # runs off the PE-critical path.  Heads go to the free dim so
            # all SBUF tensor ops are partition-0 aligned.
            den = rc_pool.tile([1, 2, QB], f32, name="den", tag="den")
            nc.vector.tensor_copy(den[:, :, :], ctx_ps[0:1, :, :])
            cx = cx_pool.tile([64, 2, QB], f32, name="cx", tag="cx")
            nc.vector.tensor_copy(cx[:, :, :], ctx_ps[HD:2 * HD, :, :])
            rc = rc_pool.tile([1, 2, QB], f32, name="rc", tag="rc")
            nc.vector.reciprocal_approx_fast(rc[:, :, :], den[:, :, :])
            rch = rc_pool.tile([1, 2, QB], bf16, name="rch", tag="rch")
            nc.vector.tensor_copy(rch[:, :, :], rc[:, :, :])
            rb = rb_pool.tile([64, 2, QB], bf16, name="rb", tag="rb")
            nc.gpsimd.partition_broadcast(rb[:, :, :], rch[:, :, :])
            stage = st_pool.tile([64, 2, QB], bf16, name="st", tag="st")
            nc.vector.tensor_mul(stage[:, :, :], cx[:, :, :], rb[:, :, :])
            # stripe qt covers dest cores 2qt (tokens 0:256) and 2qt+1
            for half in range(2):
                d = 2 * qt + half
                for j2 in range(2):
                    c0 = half * 256 + j2 * 128
                    for h in range(2):
                        nc.gpsimd.dma_start(
                            cc_in[m][b][j2][d, h * 64:(h + 1) * 64, :],
                            stage[:, h, c0:c0 + 128])

        pipe_prev = [None]

        def emit_job(job):
            e_t = emit_scores(job)
            if pipe_prev[0] is not None:
                emit_ctx(*pipe_prev[0])
            pipe_prev[0] = (job, e_t)

        def flush_jobs():
            if pipe_prev[0] is not None:
                emit_ctx(*pipe_prev[0])
                pipe_prev[0] = None

        def emit_group(m, b, qt):
            nkt = (qt + 1) * (QB // KB)
            for kt in range(nkt):
                emit_job((m, b, qt, kt, nkt))

        def emit_a2a(m, b):
            for j in range(2):
                nc.gpsimd.collective_compute(
                    "AllToAll", mybir.AluOpType.bypass,
                    replica_groups=[list(range(NCORES))],
                    ins=[cc_in[m][b][j][:, :, :]],
                    outs=[cc_out[m][b][j][:, :, :]])
                nc.sync.dma_start(
                    ct[m][b][j][:, :, :],
                    cc_out[m][b][j].rearrange("s p n -> p s n"))

        # out-projection chunk (b, j): 128 tokens x full E.  o-blocks are
        # processed in pairs sharing each LDWEIGHTS (interleaved
        # accumulation into two psum banks) so weight loads hide under the
        # previous matmul's 512 moving columns.
        def _pass_mms(m, b, j, op, jpc=1):
            o_ps = pass_ps[0].tile([128, 2, 512], f32, name="ops",
                                   tag="pass")
            for d in range(NCORES):
                for i in range(2):
                    nc.tensor.matmul(
                        o_ps[:, i, :],
                        ct[m][b][j][:, d, :],
                        wo_sb[m][:, d, (2 * op + i) * 512:
                                 (2 * op + i + 1) * 512],
                        start=(d == 0), stop=(d == NCORES - 1))
                if d % 2 == 1:
                    feed(jpc)
            return o_ps

        def emit_pass1(b, j, jpc=1):
            pp = pp_pool.tile([128, E], bf16, name="pp", tag="pp")
            for op in range(2):
                o_ps = _pass_mms(0, b, j, op, jpc)
                nc.vector.tensor_copy(
                    pp[:, 2 * op * 512:(2 * op + 2) * 512],
                    o_ps[:, :, :])
                feed(jpc)
            return pp

        def emit_pass2(b, j, pp, jpc=1):
            r0 = b * TPB + j * 128
            for op in range(2):
                o_ps = _pass_mms(1, b, j, op, jpc)
                for i in range(2):
                    o = 2 * op + i
                    ob = ob_pool.tile([128, 512], bf16, name="ob", tag="ob")
                    nc.vector.tensor_add(ob[:, :], o_ps[:, i, :],
                                         pp[:, o * 512:(o + 1) * 512])
                    nc.sync.dma_start(
                        outT[r0:r0 + 128, o * 512:(o + 1) * 512], ob[:, :])
                    feed(jpc)

        def queue_group(m, b, qt):
            nkt = (qt + 1) * (QB // KB)
            for kt in range(nkt):
                job_queue.append((m, b, qt, kt, nkt))

        # ---------------- the interleaved emission stream ----------------
        # proj chunk ch interleaved with the pair-0 group of chunk ch-1
        emit_chunk(0)
        for ch in range(1, 8):
            b, qt = divmod(ch - 1, NQT)
            queue_group(0, b, qt)
            emit_chunk(ch)
            feed(99)
            if (b, qt) == (0, NQT - 1):
                flush_jobs()
                emit_a2a(0, 0)
        emit_group(0, 1, NQT - 1)
        flush_jobs()
        proj_stack.close()
        pass_ps[0] = persist.enter_context(
            tc.tile_pool(name="pass_ps", bufs=1, space="PSUM"))
        emit_a2a(0, 1)

        # pair-1 attention interleaved with pair-0/1 out-projection
        pp_t = {}
        queue_group(1, 0, 0)
        queue_group(1, 0, 1)
        pp_t[(0, 0)] = emit_pass1(0, 0)
        pp_t[(0, 1)] = emit_pass1(0, 1)
        feed(99)
        emit_group(1, 0, 2)
        emit_group(1, 0, 3)
        flush_jobs()
        emit_a2a(1, 0)
        queue_group(1, 1, 0)
        queue_group(1, 1, 1)
        pp_t[(1, 0)] = emit_pass1(1, 0)
        pp_t[(1, 1)] = emit_pass1(1, 1)
        feed(99)
        wo_sb[1] = load_wo_half(1)
        queue_group(1, 1, 2)
        queue_group(1, 1, 3)
        emit_pass2(0, 0, pp_t[(0, 0)], jpc=2)
        feed(99)
        flush_jobs()
        emit_a2a(1, 1)
        # gap filler: batch-0 chunk, independent of the last A2A
        emit_pass2(0, 1, pp_t[(0, 1)])
        emit_pass2(1, 0, pp_t[(1, 0)])
        emit_pass2(1, 1, pp_t[(1, 1)])

    nc.compile()
    return nc


def make_in_maps(cfg, x, cos, sin, Wq, Wk, Wv, Wo):
    """Host-side prep: transpose/slice full inputs into per-core maps."""
    import ml_dtypes
    B, S, E = cfg["B"], cfg["S"], cfg["E"]
    NH, NKV, HD, NCORES = cfg["NH"], cfg["NKV"], cfg["HD"], cfg["ncores"]
    HPC = NH // NCORES
    QH = HPC * HD
    KVPC = NKV // NCORES
    bf = ml_dtypes.bfloat16

    x = np.asarray(x, dtype=np.float32)
    cos = np.asarray(cos, dtype=np.float32)
    sin = np.asarray(sin, dtype=np.float32)
    Wq = np.asarray(Wq, dtype=np.float32)
    Wk = np.asarray(Wk, dtype=np.float32)
    Wv = np.asarray(Wv, dtype=np.float32)
    Wo = np.asarray(Wo, dtype=np.float32)

    xT = np.ascontiguousarray(x.reshape(B * S, E).T.astype(bf))
    cos_t = cos.T[:HD]                        # [64, S]
    cosT = np.ascontiguousarray(
        np.concatenate([cos_t, cos_t], axis=0).astype(bf))
    sin_t = sin.T[:HD].copy()
    sin_t[:HD // 2] *= -1.0                   # signed sin for rotate-half
    sinT = np.ascontiguousarray(
        np.concatenate([sin_t, sin_t], axis=0).astype(bf))
    # Wo^T with rows re-blocked: woT_blocks[m*8+d] = Wo^T rows of
    # (src core d, pair m) = global rows [(d*4+2m)*128 : +128]  (each core
    # owns 4 heads = 2 pairs of 2 heads; pair m rows = heads (4d+2m, +1))
    woT_full = Wo.T.astype(bf)                # [E_in(ctx rows), E_out]
    blocks = []
    for m in range(2):
        for d in range(NCORES):
            r0 = (d * 4 + 2 * m) * HD
            blocks.append(woT_full[r0:r0 + 2 * HD, :])
    woT = np.ascontiguousarray(np.concatenate(blocks, axis=0))

    in_maps = []
    for c in range(NCORES):
        qsl = slice(c * QH, (c + 1) * QH)
        ksl = slice(c * KVPC * HD, (c + 1) * KVPC * HD)
        wq = np.ascontiguousarray(Wq[qsl, :].T.astype(bf))
        wkv = np.ascontiguousarray(
            np.concatenate([Wk[ksl, :].T, Wv[ksl, :].T], axis=1).astype(bf))
        in_maps.append(dict(xT=xT, wqT=wq, wkvT=wkv, woT=woT,
                            cosT=cosT, sinT=sinT))
    return in_maps


def assemble_output(cfg, results):
    B, S, E = cfg["B"], cfg["S"], cfg["E"]
    NCORES = cfg["ncores"]
    TPB = S // NCORES // 2 * 2  # 256 tokens per (core, batch)
    out = np.empty((B, S, E), dtype=np.float32)
    for c in range(NCORES):
        o = np.asarray(results[c]["outT"]).astype(np.float32)
        for b in range(B):
            out[b, c * TPB:(c + 1) * TPB, :] = o[b * TPB:(b + 1) * TPB, :]
    return out


def kernel(x, mask, cos, sin, Wq, Wk, Wv, Wo):
    global LAST_RESULTS, _CACHED_NC
    _ensure_concourse()
    from concourse import bass_utils

    cfg = FULL_CFG
    if _CACHED_NC is None:
        _CACHED_NC = build_gqa(cfg)
    nc = _CACHED_NC
    in_maps = make_in_maps(cfg, x, cos, sin, Wq, Wk, Wv, Wo)
    res = bass_utils.run_bass_kernel_spmd(
        nc, in_maps, core_ids=list(range(cfg["ncores"])))
    LAST_RESULTS = res
    return assemble_output(cfg, res.results)


# revision 25
# speedup vs baseline: 1.0166x; 1.0166x over previous
"""GroupedQueryAttention TRN2 Bass kernel (v3).

Strategy (8 NeuronCores, tensor-parallel over heads):
  - Each core owns 4 q-heads (one kv head, GQA group of 4), all tokens.
  - Single fully-interleaved emission stream so the PE never idles (keeps
    the HAM clock gate at 2.4 GHz) and exp (ACT) overlaps matmuls:
      proj chunk 0..7 (QKV + RoPE)  interleaved with  pair-0 attention
      stripes as their token chunks complete; pair-1 attention interleaved
      with pair-0 out-projection chunks; tail = last 2 small A2A pieces +
      2 out-proj chunks.
  - Attention per (head-pair, batch, q-stripe, k-block): S = K^T Q for 2
    heads, exp on ACT, ctx^T += V_aug^T exp (ones column gives the softmax
    denominator), normalize via reciprocal + partition broadcast.
  - Re-shard head->token via 16 small AllToAlls: one per (pair, batch,
    token-half-of-256) = 256 KB each, fired as soon as that (pair, batch)
    finishes.  Core c owns tokens [c*256,(c+1)*256) of EACH batch.
  - Out-proj: two passes (pair-0 into bf16 partial, pair-1 adds), emitted
    per 128-token chunk right after its A2A piece; Wo streamed in two
    4 MB halves through one SBUF slot.
"""

import os
import sys

import numpy as np


def _ensure_concourse():
    try:
        import concourse.bass  # noqa: F401
    except ImportError:
        for p in ("/opt/trn_rl_repo", "/root/.axon_site/_ro/trn_rl_repo"):
            if os.path.isdir(p) and p not in sys.path:
                sys.path.insert(0, p)
        import concourse.bass  # noqa: F401


FULL_CFG = dict(B=2, S=2048, E=2048, NH=32, NKV=8, HD=64, ncores=8, IC=512)

LAST_RESULTS = None
_CACHED_NC = None


def build_gqa(cfg):
    """Build the Bass module for one core's SPMD program. Returns nc."""
    _ensure_concourse()
    from contextlib import ExitStack

    import concourse.mybir as mybir
    import concourse.tile as tile
    from concourse import bacc
    from concourse.masks import make_identity

    dt = mybir.dt
    f32 = dt.float32
    bf16 = dt.bfloat16
    Exp = mybir.ActivationFunctionType.Exp

    B, S, E = cfg["B"], cfg["S"], cfg["E"]
    NH, NKV, HD = cfg["NH"], cfg["NKV"], cfg["HD"]
    NCORES = cfg["ncores"]
    HPC = NH // NCORES          # 4 q heads per core
    assert HPC == 4 and HD == 64
    QH = HPC * HD               # 256 q rows per core
    KVD = 2 * HD                # 128 packed K|V projection width
    NI = B * S                  # 4096 tokens
    ET = E // 128               # 16 contraction tiles
    IC = cfg["IC"]              # phase-1 token chunk (512)
    QB = 512                    # attention q stripe
    KB = 128                    # attention k block
    NQT = S // QB               # 4 stripes per batch
    SKT = S // KB               # 16 k tiles per batch
    NKTILES = NI // KB          # 32 k tiles
    TPB = 256                   # output tokens per (core, batch)
    TOK = B * TPB               # 512 output tokens per core
    scale = 1.0 / float(np.sqrt(HD))

    nc = bacc.Bacc("TRN2", target_bir_lowering=False, debug=False,
                   num_devices=NCORES)

    xT = nc.dram_tensor("xT", [E, NI], bf16, kind="ExternalInput").ap()
    wqT = nc.dram_tensor("wqT", [E, QH], bf16, kind="ExternalInput").ap()
    wkvT = nc.dram_tensor("wkvT", [E, KVD], bf16, kind="ExternalInput").ap()
    # host pre-interleaved: rows [m*1024 + d*128 + :128] = Wo^T block for
    # (src core d, head-pair m)
    woT = nc.dram_tensor("woT", [E, E], bf16, kind="ExternalInput").ap()
    cosT = nc.dram_tensor("cosT", [128, S], bf16, kind="ExternalInput").ap()
    sinT = nc.dram_tensor("sinT", [128, S], bf16, kind="ExternalInput").ap()
    # rows [b*TPB + j*128 + :128] = batch b, own tokens [j*128:(j+1)*128)
    outT = nc.dram_tensor("outT", [TOK, E], bf16, kind="ExternalOutput").ap()

    with tile.TileContext(nc) as tc, ExitStack() as persist:
        const = persist.enter_context(tc.tile_pool(name="const", bufs=1))
        qt_pool = persist.enter_context(tc.tile_pool(name="qt", bufs=1))
        kt_pool = persist.enter_context(tc.tile_pool(name="kt", bufs=1))
        vaug_pool = persist.enter_context(tc.tile_pool(name="vaug", bufs=1))
        wo_pool = persist.enter_context(tc.tile_pool(name="wo", bufs=1))
        ct_pool = persist.enter_context(tc.tile_pool(name="ct", bufs=1))
        pp_pool = persist.enter_context(tc.tile_pool(name="pp", bufs=4))
        dram = persist.enter_context(
            tc.tile_pool(name="dram", bufs=1, space="DRAM"))

        ident = const.tile([128, 128], bf16, name="ident", tag="ident")
        make_identity(nc, ident[:, :])
        # startup DMAs spread across rings: wq + x chunks on sync; cos/sin
        # then wo-half-0 on scalar; wkv on vector.
        wq_sb = const.tile([128, ET, QH], bf16, name="wq_sb", tag="wq")
        for ts in range(0, ET, 4):
            nc.scalar.dma_start(
                wq_sb[:, ts:ts + 4, :],
                wqT[ts * 128:(ts + 4) * 128, :].rearrange(
                    "(t p) o -> p t o", p=128))
        wkv_sb = const.tile([128, ET, KVD], bf16, name="wkv_sb", tag="wkv")
        nc.gpsimd.dma_start(wkv_sb[:, :, :],
                            wkvT.rearrange("(t p) o -> p t o", p=128))
        cos_sb = const.tile([128, S], bf16, name="cos_sb", tag="cos")
        nc.scalar.dma_start(cos_sb[:, :], cosT)
        sin_sb = const.tile([128, S], bf16, name="sin_sb", tag="sin")
        nc.scalar.dma_start(sin_sb[:, :], sinT)

        def load_wo_half(m):
            wo_sb = wo_pool.tile([128, NCORES, E], bf16, name=f"wo{m}",
                                 tag="wo")
            for d in range(NCORES):
                r0 = (m * NCORES + d) * 128
                nc.scalar.dma_start(wo_sb[:, d, :], woT[r0:r0 + 128, :])
            return wo_sb

        wo_sb = [None, None]
        wo_sb[0] = load_wo_half(0)

        # triangular causal mask for the diagonal 128-block, dup for 2 heads
        tri = const.tile([128, 2, 128], bf16, name="tri", tag="tri")
        nc.gpsimd.memset(tri[:, :, :], 1.0)
        nc.gpsimd.affine_select(
            out=tri[:, :, :], in_=tri[:, :, :],
            pattern=[[0, 2], [1, 128]], compare_op=mybir.AluOpType.is_ge,
            fill=0.0, base=0, channel_multiplier=-1)

        # persistent activations
        qt_sb = qt_pool.tile([64, HPC, NI], bf16, name="qt", tag="qt")
        kt_sb = kt_pool.tile([64, NI], bf16, name="kt", tag="kt")
        vaug_all = vaug_pool.tile([128, NKTILES, 2 * HD], bf16,
                                  name="vaug", tag="vaug")
        nc.vector.memset(vaug_all[:, :, :], 0.0)
        nc.vector.memset(vaug_all[:, :, 0:1], 1.0)
        vaug = [vaug_all[:, k, :] for k in range(NKTILES)]

        # collective buffers: per (pair m, batch b, token-half j):
        # [dest core, 128 rows, 128 tokens] = 256 KB
        cc_in = [[[dram.tile([NCORES, 128, 128], bf16,
                             name=f"cci{m}{b}{j}", tag=f"cci{m}{b}{j}")
                   for j in range(2)] for b in range(B)] for m in range(2)]
        cc_out = [[[dram.tile([NCORES, 128, 128], bf16,
                              name=f"cco{m}{b}{j}", tag=f"cco{m}{b}{j}")
                    for j in range(2)] for b in range(B)] for m in range(2)]
        # re-sharded ctx: [128 rows, src core, 128 tokens] per (m, b, j)
        ct = [[[ct_pool.tile([128, NCORES, 128], bf16,
                             name=f"ct{m}{b}{j}", tag=f"ct{m}{b}{j}")
                for j in range(2)] for b in range(B)] for m in range(2)]

        # ---------------- pools for the interleaved stream ----------------
        xt_pool = persist.enter_context(tc.tile_pool(name="xt", bufs=2))
        rope_pool = persist.enter_context(tc.tile_pool(name="rope", bufs=2))
        vs_pool = persist.enter_context(tc.tile_pool(name="vs", bufs=1))
        scores_ps = persist.enter_context(
            tc.tile_pool(name="scores_ps", bufs=2, space="PSUM"))
        ctx_ps_pool = persist.enter_context(
            tc.tile_pool(name="ctx_ps", bufs=1, space="PSUM"))
        et_pool = persist.enter_context(tc.tile_pool(name="et", bufs=3))
        rc_pool = persist.enter_context(tc.tile_pool(name="rc", bufs=1))
        cx_pool = persist.enter_context(tc.tile_pool(name="cx", bufs=1))
        rb_pool = persist.enter_context(tc.tile_pool(name="rb", bufs=1))
        st_pool = persist.enter_context(tc.tile_pool(name="st", bufs=2))
        ob_pool = persist.enter_context(tc.tile_pool(name="ob", bufs=2))
        proj_ps_pool = persist.enter_context(
            tc.tile_pool(name="proj_ps", bufs=2, space="PSUM"))

        # persistent band tiles with pre-zeroed garbage region so band
        # exps write only [q0:] and ctx can read the full tile
        e_band = [et_pool.tile([128, 2, QB], bf16, name=f"eb{j}",
                               tag=f"eb{j}", bufs=1)
                  for j in range(QB // KB)]
        for j in range(1, QB // KB):
            nc.gpsimd.memset(e_band[j][:, :, 0:j * KB], 0.0)

        # ---------------- emit helpers ----------------
        def rope(src_ps, parts, s0, dsts):
            # rotate-half via partition-shifted PSUM reads (PSUM operands
            # are exempt from the SBUF same-start-partition rule), so no
            # swap copies are needed and ACT stays exp-only.
            t1 = rope_pool.tile([128, IC], bf16, name="t1", tag="t1")
            sw = rope_pool.tile([128, IC], bf16, name="sw", tag="sw")
            nc.vector.tensor_mul(t1[:parts, :], src_ps[:parts, :],
                                 cos_sb[:parts, s0:s0 + IC])
            for h0 in range(0, parts, 64):
                nc.vector.tensor_mul(sw[h0:h0 + 32, :],
                                     src_ps[h0 + 32:h0 + 64, :],
                                     sin_sb[h0:h0 + 32, s0:s0 + IC])
                nc.vector.tensor_mul(sw[h0 + 32:h0 + 64, :],
                                     src_ps[h0:h0 + 32, :],
                                     sin_sb[h0 + 32:h0 + 64, s0:s0 + IC])
            for out_ap, r0 in dsts:
                nc.vector.tensor_add(out_ap, t1[r0:r0 + 64, :],
                                     sw[r0:r0 + 64, :])

        # pending attention jobs, drained one-at-a-time between matmul
        # sub-chains so the ACT exp stream never starves while the PE works
        # through projection / out-projection chains (and vice versa)
        job_queue = []

        def feed(n=1):
            for _ in range(min(n, len(job_queue))):
                emit_job(job_queue.pop(0))

        def emit_chunk(ch):
            i0 = ch * IC
            s0 = i0 % S
            xt = xt_pool.tile([128, ET, IC], bf16, name="xt", tag="xt")
            for ts in range(0, ET, 4):
                nc.sync.dma_start(
                    xt[:, ts:ts + 4, :],
                    xT[ts * 128:(ts + 4) * 128, i0:i0 + IC].rearrange(
                        "(t p) i -> p t i", p=128))
            for m in range(2):
                q_ps = proj_ps_pool.tile([128, IC], f32, name="pps",
                                         tag="proj")
                for t in range(ET):
                    nc.tensor.matmul(
                        q_ps[:, :],
                        wq_sb[:, t, m * 128:(m + 1) * 128],
                        xt[:, t, :],
                        start=(t == 0), stop=(t == ET - 1))
                    if t % 4 == 3:
                        feed()
                rope(q_ps, 128, s0,
                     [(qt_sb[0:64, 2 * m, i0:i0 + IC], 0),
                      (qt_sb[0:64, 2 * m + 1, i0:i0 + IC], 64)])
                feed()
            kv_ps = proj_ps_pool.tile([128, IC], f32, name="pps", tag="proj")
            for t in range(ET):
                nc.tensor.matmul(
                    kv_ps[:, :],
                    wkv_sb[:, t, :],
                    xt[:, t, :],
                    start=(t == 0), stop=(t == ET - 1))
                if t % 4 == 3:
                    feed()
            rope(kv_ps, 64, s0, [(kt_sb[0:64, i0:i0 + IC], 0)])
            vs = vs_pool.tile([64, IC], bf16, name="vs", tag="vs")
            nc.vector.tensor_copy(vs[:, :], kv_ps[64:128, :])
            feed()
            for j in range(IC // 128):
                kidx = (i0 + j * 128) // 128
                vt_ps = proj_ps_pool.tile([128, HD], bf16, name="vt",
                                          tag="proj")
                nc.tensor.transpose(vt_ps[:, :],
                                    vs[:, j * 128:(j + 1) * 128],
                                    ident[0:64, 0:64])
                nc.vector.tensor_copy(vaug[kidx][:, HD:2 * HD], vt_ps[:, :])
                feed()

        # ---------------- attention ----------------
        def emit_scores(job):
            m, b, qt, kt, nkt = job
            sl = b * S + qt * QB
            j = kt - qt * (QB // KB)
            kp = b * S + kt * KB
            s_ps = scores_ps.tile([128, 2, QB], f32, name="sps", tag="sps")
            if j < 0:
                e_t = et_pool.tile([128, 2, QB], bf16, name="et", tag="et")
                for h in range(2):
                    nc.tensor.matmul(
                        s_ps[:, h, :],
                        kt_sb[0:64, kp:kp + KB],
                        qt_sb[0:64, 2 * m + h, sl:sl + QB],
                        start=True, stop=True)
                nc.scalar.activation(e_t[:, :, :], s_ps[:, :, :], Exp,
                                     scale=scale)
            else:
                e_t = e_band[j]
                q0 = j * KB
                for h in range(2):
                    nc.tensor.matmul(
                        s_ps[:, h, q0:QB],
                        kt_sb[0:64, kp:kp + KB],
                        qt_sb[0:64, 2 * m + h, sl + q0:sl + QB],
                        start=True, stop=True)
                nc.scalar.activation(e_t[:, :, q0:QB],
                                     s_ps[:, :, q0:QB], Exp,
                                     scale=scale)
                nc.vector.tensor_mul(e_t[:, :, q0:q0 + KB],
                                     e_t[:, :, q0:q0 + KB],
                                     tri[:, :, :])
            return e_t

        ctx_cur = [None]

        def emit_ctx(job, e_t):
            m, b, qt, kt, nkt = job
            if kt == 0:
                ctx_cur[0] = ctx_ps_pool.tile([128, 2, QB], f32,
                                              name="ctx", tag="ctx")
            ctx_ps = ctx_cur[0]
            j = kt - qt * (QB // KB)
            q0 = max(j, 0) * KB  # zero region of band tiles: skip it
            for h in range(2):
                nc.tensor.matmul(
                    ctx_ps[:, h, q0:QB],
                    vaug[b * SKT + kt][:, :],
                    e_t[:, h, q0:QB],
                    start=(kt == 0), stop=(kt == nkt - 1),
                    skip_group_check=(q0 > 0))
            if kt != nkt - 1:
                return
            # decouple: two fast copies free the ctx psum bank for the next
            # stripe; the normalize chain (recip -> broadcast -> mul) then
            # runs off the PE-critical path.  Heads go to the free dim so
            # all SBUF tensor ops are partition-0 aligned.
            den = rc_pool.tile([1, 2, QB], f32, name="den", tag="den")
            nc.vector.tensor_copy(den[:, :, :], ctx_ps[0:1, :, :])
            cx = cx_pool.tile([64, 2, QB], f32, name="cx", tag="cx")
            nc.vector.tensor_copy(cx[:, :, :], ctx_ps[HD:2 * HD, :, :])
            rc = rc_pool.tile([1, 2, QB], f32, name="rc", tag="rc")
            nc.vector.reciprocal_approx_fast(rc[:, :, :], den[:, :, :])
            rch = rc_pool.tile([1, 2, QB], bf16, name="rch", tag="rch")
            nc.vector.tensor_copy(rch[:, :, :], rc[:, :, :])
            rb = rb_pool.tile([64, 2, QB], bf16, name="rb", tag="rb")
            nc.gpsimd.partition_broadcast(rb[:, :, :], rch[:, :, :])
            stage = st_pool.tile([64, 2, QB], bf16, name="st", tag="st")
            nc.vector.tensor_mul(stage[:, :, :], cx[:, :, :], rb[:, :, :])
            # stripe qt covers dest cores 2qt (tokens 0:256) and 2qt+1
            for half in range(2):
                d = 2 * qt + half
                for j2 in range(2):
                    c0 = half * 256 + j2 * 128
                    for h in range(2):
                        nc.gpsimd.dma_start(
                            cc_in[m][b][j2][d, h * 64:(h + 1) * 64, :],
                            stage[:, h, c0:c0 + 128])
            if m == 0 and qt == NQT - 1:
                emit_a2a(0, b)

        pipe_prev = [None]

        def emit_job(job):
            e_t = emit_scores(job)
            if pipe_prev[0] is not None:
                emit_ctx(*pipe_prev[0])
            pipe_prev[0] = (job, e_t)

        def flush_jobs():
            if pipe_prev[0] is not None:
                emit_ctx(*pipe_prev[0])
                pipe_prev[0] = None

        def emit_group(m, b, qt):
            nkt = (qt + 1) * (QB // KB)
            for kt in range(nkt):
                emit_job((m, b, qt, kt, nkt))

        def emit_a2a(m, b):
            for j in range(2):
                nc.gpsimd.collective_compute(
                    "AllToAll", mybir.AluOpType.bypass,
                    replica_groups=[list(range(NCORES))],
                    ins=[cc_in[m][b][j][:, :, :]],
                    outs=[cc_out[m][b][j][:, :, :]])
                nc.sync.dma_start(
                    ct[m][b][j][:, :, :],
                    cc_out[m][b][j].rearrange("s p n -> p s n"))

        # out-projection chunk (b, j): 128 tokens x full E.  o-blocks are
        # processed in pairs sharing each LDWEIGHTS (interleaved
        # accumulation into two psum banks) so weight loads hide under the
        # previous matmul's 512 moving columns.
        def _pass_mms(m, b, j, op, jpc=1):
            o_ps = scores_ps.tile([128, 2, 512], f32, name="ops",
                                  tag="sps")
            for d in range(NCORES):
                for i in range(2):
                    nc.tensor.matmul(
                        o_ps[:, i, :],
                        ct[m][b][j][:, d, :],
                        wo_sb[m][:, d, (2 * op + i) * 512:
                                 (2 * op + i + 1) * 512],
                        start=(d == 0), stop=(d == NCORES - 1))
                if d % 2 == 1:
                    feed(jpc)
            return o_ps

        def emit_pass1(b, j, jpc=1):
            pp = pp_pool.tile([128, E], bf16, name="pp", tag="pp")
            for op in range(2):
                o_ps = _pass_mms(0, b, j, op, jpc)
                nc.vector.tensor_copy(
                    pp[:, 2 * op * 512:(2 * op + 2) * 512],
                    o_ps[:, :, :])
                feed(jpc)
            return pp

        def emit_pass2(b, j, pp, jpc=1):
            r0 = b * TPB + j * 128
            for op in range(2):
                o_ps = _pass_mms(1, b, j, op, jpc)
                for i in range(2):
                    o = 2 * op + i
                    ob = ob_pool.tile([128, 512], bf16, name="ob", tag="ob")
                    nc.vector.tensor_add(ob[:, :], o_ps[:, i, :],
                                         pp[:, o * 512:(o + 1) * 512])
                    nc.sync.dma_start(
                        outT[r0:r0 + 128, o * 512:(o + 1) * 512], ob[:, :])
                    feed(jpc)

        def queue_group(m, b, qt):
            nkt = (qt + 1) * (QB // KB)
            for kt in range(nkt):
                job_queue.append((m, b, qt, kt, nkt))

        # ---------------- the interleaved emission stream ----------------
        # proj chunk ch interleaved with pair-0 attention; unfed jobs carry
        # over across chunk boundaries so the PE never drains its filler.
        # Pair-0 A2As fire from emit_ctx as their (pair,batch) completes.
        emit_chunk(0)
        for ch in range(1, 8):
            b, qt = divmod(ch - 1, NQT)
            queue_group(0, b, qt)
            emit_chunk(ch)
        queue_group(0, 1, NQT - 1)
        feed(99)
        flush_jobs()

        # pair-1 attention interleaved with pair-0/1 out-projection
        pp_t = {}
        queue_group(1, 0, 0)
        queue_group(1, 0, 1)
        pp_t[(0, 0)] = emit_pass1(0, 0)
        pp_t[(0, 1)] = emit_pass1(0, 1)
        feed(99)
        emit_group(1, 0, 2)
        emit_group(1, 0, 3)
        flush_jobs()
        emit_a2a(1, 0)
        queue_group(1, 1, 0)
        queue_group(1, 1, 1)
        pp_t[(1, 0)] = emit_pass1(1, 0)
        pp_t[(1, 1)] = emit_pass1(1, 1)
        feed(99)
        wo_sb[1] = load_wo_half(1)
        queue_group(1, 1, 2)
        queue_group(1, 1, 3)
        emit_pass2(0, 0, pp_t[(0, 0)], jpc=2)
        feed(99)
        flush_jobs()
        emit_a2a(1, 1)
        # gap filler: batch-0 chunk, independent of the last A2A
        emit_pass2(0, 1, pp_t[(0, 1)])
        emit_pass2(1, 0, pp_t[(1, 0)])
        emit_pass2(1, 1, pp_t[(1, 1)])

    nc.compile()
    return nc


def make_in_maps(cfg, x, cos, sin, Wq, Wk, Wv, Wo):
    """Host-side prep: transpose/slice full inputs into per-core maps."""
    import ml_dtypes
    B, S, E = cfg["B"], cfg["S"], cfg["E"]
    NH, NKV, HD, NCORES = cfg["NH"], cfg["NKV"], cfg["HD"], cfg["ncores"]
    HPC = NH // NCORES
    QH = HPC * HD
    KVPC = NKV // NCORES
    bf = ml_dtypes.bfloat16

    x = np.asarray(x, dtype=np.float32)
    cos = np.asarray(cos, dtype=np.float32)
    sin = np.asarray(sin, dtype=np.float32)
    Wq = np.asarray(Wq, dtype=np.float32)
    Wk = np.asarray(Wk, dtype=np.float32)
    Wv = np.asarray(Wv, dtype=np.float32)
    Wo = np.asarray(Wo, dtype=np.float32)

    xT = np.ascontiguousarray(x.reshape(B * S, E).T.astype(bf))
    cos_t = cos.T[:HD]                        # [64, S]
    cosT = np.ascontiguousarray(
        np.concatenate([cos_t, cos_t], axis=0).astype(bf))
    sin_t = sin.T[:HD].copy()
    sin_t[:HD // 2] *= -1.0                   # signed sin for rotate-half
    sinT = np.ascontiguousarray(
        np.concatenate([sin_t, sin_t], axis=0).astype(bf))
    # Wo^T with rows re-blocked: woT_blocks[m*8+d] = Wo^T rows of
    # (src core d, pair m) = global rows [(d*4+2m)*128 : +128]  (each core
    # owns 4 heads = 2 pairs of 2 heads; pair m rows = heads (4d+2m, +1))
    woT_full = Wo.T.astype(bf)                # [E_in(ctx rows), E_out]
    blocks = []
    for m in range(2):
        for d in range(NCORES):
            r0 = (d * 4 + 2 * m) * HD
            blocks.append(woT_full[r0:r0 + 2 * HD, :])
    woT = np.ascontiguousarray(np.concatenate(blocks, axis=0))

    in_maps = []
    for c in range(NCORES):
        qsl = slice(c * QH, (c + 1) * QH)
        ksl = slice(c * KVPC * HD, (c + 1) * KVPC * HD)
        wq = np.ascontiguousarray(Wq[qsl, :].T.astype(bf))
        wkv = np.ascontiguousarray(
            np.concatenate([Wk[ksl, :].T, Wv[ksl, :].T], axis=1).astype(bf))
        in_maps.append(dict(xT=xT, wqT=wq, wkvT=wkv, woT=woT,
                            cosT=cosT, sinT=sinT))
    return in_maps


def assemble_output(cfg, results):
    B, S, E = cfg["B"], cfg["S"], cfg["E"]
    NCORES = cfg["ncores"]
    TPB = S // NCORES // 2 * 2  # 256 tokens per (core, batch)
    out = np.empty((B, S, E), dtype=np.float32)
    for c in range(NCORES):
        o = np.asarray(results[c]["outT"]).astype(np.float32)
        for b in range(B):
            out[b, c * TPB:(c + 1) * TPB, :] = o[b * TPB:(b + 1) * TPB, :]
    return out


def kernel(x, mask, cos, sin, Wq, Wk, Wv, Wo):
    global LAST_RESULTS, _CACHED_NC
    _ensure_concourse()
    from concourse import bass_utils

    cfg = FULL_CFG
    if _CACHED_NC is None:
        _CACHED_NC = build_gqa(cfg)
    nc = _CACHED_NC
    in_maps = make_in_maps(cfg, x, cos, sin, Wq, Wk, Wv, Wo)
    res = bass_utils.run_bass_kernel_spmd(
        nc, in_maps, core_ids=list(range(cfg["ncores"])))
    LAST_RESULTS = res
    return assemble_output(cfg, res.results)


# revision 26
# speedup vs baseline: 1.0191x; 1.0025x over previous
"""GroupedQueryAttention TRN2 Bass kernel (v3).

Strategy (8 NeuronCores, tensor-parallel over heads):
  - Each core owns 4 q-heads (one kv head, GQA group of 4), all tokens.
  - Single fully-interleaved emission stream so the PE never idles (keeps
    the HAM clock gate at 2.4 GHz) and exp (ACT) overlaps matmuls:
      proj chunk 0..7 (QKV + RoPE)  interleaved with  pair-0 attention
      stripes as their token chunks complete; pair-1 attention interleaved
      with pair-0 out-projection chunks; tail = last 2 small A2A pieces +
      2 out-proj chunks.
  - Attention per (head-pair, batch, q-stripe, k-block): S = K^T Q for 2
    heads, exp on ACT, ctx^T += V_aug^T exp (ones column gives the softmax
    denominator), normalize via reciprocal + partition broadcast.
  - Re-shard head->token via 16 small AllToAlls: one per (pair, batch,
    token-half-of-256) = 256 KB each, fired as soon as that (pair, batch)
    finishes.  Core c owns tokens [c*256,(c+1)*256) of EACH batch.
  - Out-proj: two passes (pair-0 into bf16 partial, pair-1 adds), emitted
    per 128-token chunk right after its A2A piece; Wo streamed in two
    4 MB halves through one SBUF slot.
"""

import os
import sys

import numpy as np


def _ensure_concourse():
    try:
        import concourse.bass  # noqa: F401
    except ImportError:
        for p in ("/opt/trn_rl_repo", "/root/.axon_site/_ro/trn_rl_repo"):
            if os.path.isdir(p) and p not in sys.path:
                sys.path.insert(0, p)
        import concourse.bass  # noqa: F401


FULL_CFG = dict(B=2, S=2048, E=2048, NH=32, NKV=8, HD=64, ncores=8, IC=512)

LAST_RESULTS = None
_CACHED_NC = None


def build_gqa(cfg):
    """Build the Bass module for one core's SPMD program. Returns nc."""
    _ensure_concourse()
    from contextlib import ExitStack

    import concourse.mybir as mybir
    import concourse.tile as tile
    from concourse import bacc
    from concourse.masks import make_identity

    dt = mybir.dt
    f32 = dt.float32
    bf16 = dt.bfloat16
    Exp = mybir.ActivationFunctionType.Exp

    B, S, E = cfg["B"], cfg["S"], cfg["E"]
    NH, NKV, HD = cfg["NH"], cfg["NKV"], cfg["HD"]
    NCORES = cfg["ncores"]
    HPC = NH // NCORES          # 4 q heads per core
    assert HPC == 4 and HD == 64
    QH = HPC * HD               # 256 q rows per core
    KVD = 2 * HD                # 128 packed K|V projection width
    NI = B * S                  # 4096 tokens
    ET = E // 128               # 16 contraction tiles
    IC = cfg["IC"]              # phase-1 token chunk (512)
    QB = 512                    # attention q stripe
    KB = 128                    # attention k block
    NQT = S // QB               # 4 stripes per batch
    SKT = S // KB               # 16 k tiles per batch
    NKTILES = NI // KB          # 32 k tiles
    TPB = 256                   # output tokens per (core, batch)
    TOK = B * TPB               # 512 output tokens per core
    scale = 1.0 / float(np.sqrt(HD))

    nc = bacc.Bacc("TRN2", target_bir_lowering=False, debug=False,
                   num_devices=NCORES)

    xT = nc.dram_tensor("xT", [E, NI], bf16, kind="ExternalInput").ap()
    wqT = nc.dram_tensor("wqT", [E, QH], bf16, kind="ExternalInput").ap()
    wkvT = nc.dram_tensor("wkvT", [E, KVD], bf16, kind="ExternalInput").ap()
    # host pre-interleaved: rows [m*1024 + d*128 + :128] = Wo^T block for
    # (src core d, head-pair m)
    woT = nc.dram_tensor("woT", [E, E], bf16, kind="ExternalInput").ap()
    cosT = nc.dram_tensor("cosT", [128, S], bf16, kind="ExternalInput").ap()
    sinT = nc.dram_tensor("sinT", [128, S], bf16, kind="ExternalInput").ap()
    # rows [b*TPB + j*128 + :128] = batch b, own tokens [j*128:(j+1)*128)
    outT = nc.dram_tensor("outT", [TOK, E], bf16, kind="ExternalOutput").ap()

    with tile.TileContext(nc) as tc, ExitStack() as persist:
        const = persist.enter_context(tc.tile_pool(name="const", bufs=1))
        qt_pool = persist.enter_context(tc.tile_pool(name="qt", bufs=1))
        kt_pool = persist.enter_context(tc.tile_pool(name="kt", bufs=1))
        vaug_pool = persist.enter_context(tc.tile_pool(name="vaug", bufs=1))
        wo_pool = persist.enter_context(tc.tile_pool(name="wo", bufs=1))
        ct_pool = persist.enter_context(tc.tile_pool(name="ct", bufs=1))
        pp_pool = persist.enter_context(tc.tile_pool(name="pp", bufs=4))
        dram = persist.enter_context(
            tc.tile_pool(name="dram", bufs=1, space="DRAM"))

        ident = const.tile([128, 128], bf16, name="ident", tag="ident")
        make_identity(nc, ident[:, :])
        # startup DMAs spread across rings: wq + x chunks on sync; cos/sin
        # then wo-half-0 on scalar; wkv on vector.
        wq_sb = const.tile([128, ET, QH], bf16, name="wq_sb", tag="wq")
        for ts in range(0, ET, 4):
            nc.scalar.dma_start(
                wq_sb[:, ts:ts + 4, :],
                wqT[ts * 128:(ts + 4) * 128, :].rearrange(
                    "(t p) o -> p t o", p=128))
        wkv_sb = const.tile([128, ET, KVD], bf16, name="wkv_sb", tag="wkv")
        nc.gpsimd.dma_start(wkv_sb[:, :, :],
                            wkvT.rearrange("(t p) o -> p t o", p=128))
        cos_sb = const.tile([128, S], bf16, name="cos_sb", tag="cos")
        nc.scalar.dma_start(cos_sb[:, :], cosT)
        sin_sb = const.tile([128, S], bf16, name="sin_sb", tag="sin")
        nc.scalar.dma_start(sin_sb[:, :], sinT)

        def load_wo_half(m):
            wo_sb = wo_pool.tile([128, NCORES, E], bf16, name=f"wo{m}",
                                 tag="wo")
            for d in range(NCORES):
                r0 = (m * NCORES + d) * 128
                nc.scalar.dma_start(wo_sb[:, d, :], woT[r0:r0 + 128, :])
            return wo_sb

        wo_sb = [None, None]
        wo_sb[0] = load_wo_half(0)

        # triangular causal mask for the diagonal 128-block, dup for 2 heads
        tri = const.tile([128, 2, 128], bf16, name="tri", tag="tri")
        nc.gpsimd.memset(tri[:, :, :], 1.0)
        nc.gpsimd.affine_select(
            out=tri[:, :, :], in_=tri[:, :, :],
            pattern=[[0, 2], [1, 128]], compare_op=mybir.AluOpType.is_ge,
            fill=0.0, base=0, channel_multiplier=-1)

        # persistent activations
        qt_sb = qt_pool.tile([64, HPC, NI], bf16, name="qt", tag="qt")
        kt_sb = kt_pool.tile([64, NI], bf16, name="kt", tag="kt")
        vaug_all = vaug_pool.tile([128, NKTILES, 2 * HD], bf16,
                                  name="vaug", tag="vaug")
        nc.vector.memset(vaug_all[:, :, :], 0.0)
        nc.vector.memset(vaug_all[:, :, 0:1], 1.0)
        vaug = [vaug_all[:, k, :] for k in range(NKTILES)]

        # collective buffers: per (pair m, batch b, token-half j):
        # [dest core, 128 rows, 128 tokens] = 256 KB
        cc_in = [[[dram.tile([NCORES, 128, 128], bf16,
                             name=f"cci{m}{b}{j}", tag=f"cci{m}{b}{j}")
                   for j in range(2)] for b in range(B)] for m in range(2)]
        cc_out = [[[dram.tile([NCORES, 128, 128], bf16,
                              name=f"cco{m}{b}{j}", tag=f"cco{m}{b}{j}")
                    for j in range(2)] for b in range(B)] for m in range(2)]
        # re-sharded ctx: [128 rows, src core, 128 tokens] per (m, b, j)
        ct = [[[ct_pool.tile([128, NCORES, 128], bf16,
                             name=f"ct{m}{b}{j}", tag=f"ct{m}{b}{j}")
                for j in range(2)] for b in range(B)] for m in range(2)]

        # ---------------- pools for the interleaved stream ----------------
        xt_pool = persist.enter_context(tc.tile_pool(name="xt", bufs=2))
        rope_pool = persist.enter_context(tc.tile_pool(name="rope", bufs=2))
        vs_pool = persist.enter_context(tc.tile_pool(name="vs", bufs=1))
        scores_ps = persist.enter_context(
            tc.tile_pool(name="scores_ps", bufs=2, space="PSUM"))
        ctx_ps_pool = persist.enter_context(
            tc.tile_pool(name="ctx_ps", bufs=1, space="PSUM"))
        et_pool = persist.enter_context(tc.tile_pool(name="et", bufs=3))
        rc_pool = persist.enter_context(tc.tile_pool(name="rc", bufs=1))
        cx_pool = persist.enter_context(tc.tile_pool(name="cx", bufs=1))
        rb_pool = persist.enter_context(tc.tile_pool(name="rb", bufs=1))
        st_pool = persist.enter_context(tc.tile_pool(name="st", bufs=2))
        ob_pool = persist.enter_context(tc.tile_pool(name="ob", bufs=2))
        proj_ps_pool = persist.enter_context(
            tc.tile_pool(name="proj_ps", bufs=2, space="PSUM"))

        # persistent band tiles with pre-zeroed garbage region so band
        # exps write only [q0:] and ctx can read the full tile
        e_band = [et_pool.tile([128, 2, QB], bf16, name=f"eb{j}",
                               tag=f"eb{j}", bufs=1)
                  for j in range(QB // KB)]
        for j in range(1, QB // KB):
            nc.gpsimd.memset(e_band[j][:, :, 0:j * KB], 0.0)

        # ---------------- emit helpers ----------------
        def rope(src_ps, parts, s0, dsts):
            # rotate-half via partition-shifted PSUM reads (PSUM operands
            # are exempt from the SBUF same-start-partition rule), so no
            # swap copies are needed and ACT stays exp-only.
            t1 = rope_pool.tile([128, IC], bf16, name="t1", tag="t1")
            sw = rope_pool.tile([128, IC], bf16, name="sw", tag="sw")
            nc.vector.tensor_mul(t1[:parts, :], src_ps[:parts, :],
                                 cos_sb[:parts, s0:s0 + IC])
            for h0 in range(0, parts, 64):
                nc.vector.tensor_mul(sw[h0:h0 + 32, :],
                                     src_ps[h0 + 32:h0 + 64, :],
                                     sin_sb[h0:h0 + 32, s0:s0 + IC])
                nc.vector.tensor_mul(sw[h0 + 32:h0 + 64, :],
                                     src_ps[h0:h0 + 32, :],
                                     sin_sb[h0 + 32:h0 + 64, s0:s0 + IC])
            for out_ap, r0 in dsts:
                nc.vector.tensor_add(out_ap, t1[r0:r0 + 64, :],
                                     sw[r0:r0 + 64, :])

        # pending attention jobs, drained one-at-a-time between matmul
        # sub-chains so the ACT exp stream never starves while the PE works
        # through projection / out-projection chains (and vice versa)
        job_queue = []

        def feed(n=1):
            for _ in range(min(n, len(job_queue))):
                emit_job(job_queue.pop(0))

        def emit_chunk(ch):
            i0 = ch * IC
            s0 = i0 % S
            xt = xt_pool.tile([128, ET, IC], bf16, name="xt", tag="xt")
            for ts in range(0, ET, 4):
                nc.sync.dma_start(
                    xt[:, ts:ts + 4, :],
                    xT[ts * 128:(ts + 4) * 128, i0:i0 + IC].rearrange(
                        "(t p) i -> p t i", p=128))
            for m in range(2):
                q_ps = proj_ps_pool.tile([128, IC], f32, name="pps",
                                         tag="proj")
                for t in range(ET):
                    nc.tensor.matmul(
                        q_ps[:, :],
                        wq_sb[:, t, m * 128:(m + 1) * 128],
                        xt[:, t, :],
                        start=(t == 0), stop=(t == ET - 1))
                    if t % 4 == 3:
                        feed()
                rope(q_ps, 128, s0,
                     [(qt_sb[0:64, 2 * m, i0:i0 + IC], 0),
                      (qt_sb[0:64, 2 * m + 1, i0:i0 + IC], 64)])
                feed()
            kv_ps = proj_ps_pool.tile([128, IC], f32, name="pps", tag="proj")
            for t in range(ET):
                nc.tensor.matmul(
                    kv_ps[:, :],
                    wkv_sb[:, t, :],
                    xt[:, t, :],
                    start=(t == 0), stop=(t == ET - 1))
                if t % 4 == 3:
                    feed()
            rope(kv_ps, 64, s0, [(kt_sb[0:64, i0:i0 + IC], 0)])
            vs = vs_pool.tile([64, IC], bf16, name="vs", tag="vs")
            nc.vector.tensor_copy(vs[:, :], kv_ps[64:128, :])
            feed()
            for j in range(IC // 128):
                kidx = (i0 + j * 128) // 128
                vt_ps = proj_ps_pool.tile([128, HD], bf16, name="vt",
                                          tag="proj")
                nc.tensor.transpose(vt_ps[:, :],
                                    vs[:, j * 128:(j + 1) * 128],
                                    ident[0:64, 0:64])
                nc.vector.tensor_copy(vaug[kidx][:, HD:2 * HD], vt_ps[:, :])
                feed()

        # ---------------- attention ----------------
        def emit_scores(job):
            m, b, qt, kt, nkt = job
            sl = b * S + qt * QB
            j = kt - qt * (QB // KB)
            kp = b * S + kt * KB
            s_ps = scores_ps.tile([128, 2, QB], f32, name="sps", tag="sps")
            if j < 0:
                e_t = et_pool.tile([128, 2, QB], bf16, name="et", tag="et")
                for h in range(2):
                    nc.tensor.matmul(
                        s_ps[:, h, :],
                        kt_sb[0:64, kp:kp + KB],
                        qt_sb[0:64, 2 * m + h, sl:sl + QB],
                        start=True, stop=True)
                nc.scalar.activation(e_t[:, :, :], s_ps[:, :, :], Exp,
                                     scale=scale)
            else:
                e_t = e_band[j]
                q0 = j * KB
                for h in range(2):
                    nc.tensor.matmul(
                        s_ps[:, h, q0:QB],
                        kt_sb[0:64, kp:kp + KB],
                        qt_sb[0:64, 2 * m + h, sl + q0:sl + QB],
                        start=True, stop=True)
                nc.scalar.activation(e_t[:, :, q0:QB],
                                     s_ps[:, :, q0:QB], Exp,
                                     scale=scale)
                nc.vector.tensor_mul(e_t[:, :, q0:q0 + KB],
                                     e_t[:, :, q0:q0 + KB],
                                     tri[:, :, :])
            return e_t

        ctx_cur = [None]

        def emit_ctx(job, e_t):
            m, b, qt, kt, nkt = job
            if kt == 0:
                ctx_cur[0] = ctx_ps_pool.tile([128, 2, QB], f32,
                                              name="ctx", tag="ctx")
            ctx_ps = ctx_cur[0]
            j = kt - qt * (QB // KB)
            q0 = max(j, 0) * KB  # zero region of band tiles: skip it
            for h in range(2):
                nc.tensor.matmul(
                    ctx_ps[:, h, q0:QB],
                    vaug[b * SKT + kt][:, :],
                    e_t[:, h, q0:QB],
                    start=(kt == 0), stop=(kt == nkt - 1),
                    skip_group_check=(q0 > 0))
            if kt != nkt - 1:
                return
            # decouple: two fast copies free the ctx psum bank for the next
            # stripe; the normalize chain (recip -> broadcast -> mul) then
            # runs off the PE-critical path.  Heads go to the free dim so
            # all SBUF tensor ops are partition-0 aligned.
            den = rc_pool.tile([1, 2, QB], f32, name="den", tag="den")
            nc.vector.tensor_copy(den[:, :, :], ctx_ps[0:1, :, :])
            cx = cx_pool.tile([64, 2, QB], f32, name="cx", tag="cx")
            nc.vector.tensor_copy(cx[:, :, :], ctx_ps[HD:2 * HD, :, :])
            rc = rc_pool.tile([1, 2, QB], f32, name="rc", tag="rc")
            nc.vector.reciprocal_approx_fast(rc[:, :, :], den[:, :, :])
            rch = rc_pool.tile([1, 2, QB], bf16, name="rch", tag="rch")
            nc.vector.tensor_copy(rch[:, :, :], rc[:, :, :])
            rb = rb_pool.tile([64, 2, QB], bf16, name="rb", tag="rb")
            nc.gpsimd.partition_broadcast(rb[:, :, :], rch[:, :, :])
            stage = st_pool.tile([64, 2, QB], bf16, name="st", tag="st")
            nc.vector.tensor_mul(stage[:, :, :], cx[:, :, :], rb[:, :, :])
            # stripe qt covers dest cores 2qt (tokens 0:256) and 2qt+1
            for half in range(2):
                d = 2 * qt + half
                for j2 in range(2):
                    c0 = half * 256 + j2 * 128
                    for h in range(2):
                        nc.gpsimd.dma_start(
                            cc_in[m][b][j2][d, h * 64:(h + 1) * 64, :],
                            stage[:, h, c0:c0 + 128])
            if m == 0 and qt == NQT - 1:
                emit_a2a(0, b)

        pipe_prev = [None]

        def emit_job(job):
            e_t = emit_scores(job)
            if pipe_prev[0] is not None:
                emit_ctx(*pipe_prev[0])
            pipe_prev[0] = (job, e_t)

        def flush_jobs():
            if pipe_prev[0] is not None:
                emit_ctx(*pipe_prev[0])
                pipe_prev[0] = None

        def emit_group(m, b, qt):
            nkt = (qt + 1) * (QB // KB)
            for kt in range(nkt):
                emit_job((m, b, qt, kt, nkt))

        def emit_a2a(m, b):
            for j in range(2):
                nc.gpsimd.collective_compute(
                    "AllToAll", mybir.AluOpType.bypass,
                    replica_groups=[list(range(NCORES))],
                    ins=[cc_in[m][b][j][:, :, :]],
                    outs=[cc_out[m][b][j][:, :, :]])
                nc.sync.dma_start(
                    ct[m][b][j][:, :, :],
                    cc_out[m][b][j].rearrange("s p n -> p s n"))

        # out-projection chunk (b, j): 128 tokens x full E.  o-blocks are
        # processed in pairs sharing each LDWEIGHTS (interleaved
        # accumulation into two psum banks) so weight loads hide under the
        # previous matmul's 512 moving columns.
        def _pass_mms(m, b, j, op, jpc=1):
            o_ps = scores_ps.tile([128, 2, 512], f32, name="ops",
                                  tag="sps")
            for d in range(NCORES):
                for i in range(2):
                    nc.tensor.matmul(
                        o_ps[:, i, :],
                        ct[m][b][j][:, d, :],
                        wo_sb[m][:, d, (2 * op + i) * 512:
                                 (2 * op + i + 1) * 512],
                        start=(d == 0), stop=(d == NCORES - 1))
            return o_ps

        def emit_pass1(b, j, jpc=2):
            pp = pp_pool.tile([128, E], bf16, name="pp", tag="pp")
            for op in range(2):
                o_ps = _pass_mms(0, b, j, op, jpc)
                nc.vector.tensor_copy(
                    pp[:, 2 * op * 512:(2 * op + 2) * 512],
                    o_ps[:, :, :])
                feed(jpc)
            return pp

        def emit_pass2(b, j, pp, jpc=2):
            r0 = b * TPB + j * 128
            for op in range(2):
                o_ps = _pass_mms(1, b, j, op, jpc)
                for i in range(2):
                    o = 2 * op + i
                    ob = ob_pool.tile([128, 512], bf16, name="ob", tag="ob")
                    nc.vector.tensor_add(ob[:, :], o_ps[:, i, :],
                                         pp[:, o * 512:(o + 1) * 512])
                    nc.sync.dma_start(
                        outT[r0:r0 + 128, o * 512:(o + 1) * 512], ob[:, :])
                feed(jpc)

        def queue_group(m, b, qt):
            nkt = (qt + 1) * (QB // KB)
            for kt in range(nkt):
                job_queue.append((m, b, qt, kt, nkt))

        # ---------------- the interleaved emission stream ----------------
        # Both pairs' attention spreads across the chunk/pass stream (a
        # stripe only needs its token chunks); unfed jobs carry over so the
        # PE never drains its filler.  Pair-0 A2As fire from emit_ctx.
        emit_chunk(0)
        for ch in range(1, 8):
            b, qt = divmod(ch - 1, NQT)
            queue_group(0, b, qt)
            if ch >= 5:
                queue_group(1, 0, ch - 5)
            emit_chunk(ch)
        pp_t = {}
        queue_group(0, 1, NQT - 1)
        pp_t[(0, 0)] = emit_pass1(0, 0)
        feed(99)
        flush_jobs()                      # fires a2a(0,1) via emit_ctx
        queue_group(1, 0, 3)
        pp_t[(0, 1)] = emit_pass1(0, 1)
        feed(99)
        flush_jobs()
        emit_a2a(1, 0)
        queue_group(1, 1, 0)
        queue_group(1, 1, 1)
        pp_t[(1, 0)] = emit_pass1(1, 0)
        pp_t[(1, 1)] = emit_pass1(1, 1)
        feed(99)
        wo_sb[1] = load_wo_half(1)
        queue_group(1, 1, 2)
        queue_group(1, 1, 3)
        emit_pass2(0, 0, pp_t[(0, 0)], jpc=3)
        feed(99)
        flush_jobs()
        emit_a2a(1, 1)
        # gap filler: batch-0 chunk, independent of the last A2A
        emit_pass2(0, 1, pp_t[(0, 1)])
        emit_pass2(1, 0, pp_t[(1, 0)])
        emit_pass2(1, 1, pp_t[(1, 1)])

    nc.compile()
    return nc


def make_in_maps(cfg, x, cos, sin, Wq, Wk, Wv, Wo):
    """Host-side prep: transpose/slice full inputs into per-core maps."""
    import ml_dtypes
    B, S, E = cfg["B"], cfg["S"], cfg["E"]
    NH, NKV, HD, NCORES = cfg["NH"], cfg["NKV"], cfg["HD"], cfg["ncores"]
    HPC = NH // NCORES
    QH = HPC * HD
    KVPC = NKV // NCORES
    bf = ml_dtypes.bfloat16

    x = np.asarray(x, dtype=np.float32)
    cos = np.asarray(cos, dtype=np.float32)
    sin = np.asarray(sin, dtype=np.float32)
    Wq = np.asarray(Wq, dtype=np.float32)
    Wk = np.asarray(Wk, dtype=np.float32)
    Wv = np.asarray(Wv, dtype=np.float32)
    Wo = np.asarray(Wo, dtype=np.float32)

    xT = np.ascontiguousarray(x.reshape(B * S, E).T.astype(bf))
    cos_t = cos.T[:HD]                        # [64, S]
    cosT = np.ascontiguousarray(
        np.concatenate([cos_t, cos_t], axis=0).astype(bf))
    sin_t = sin.T[:HD].copy()
    sin_t[:HD // 2] *= -1.0                   # signed sin for rotate-half
    sinT = np.ascontiguousarray(
        np.concatenate([sin_t, sin_t], axis=0).astype(bf))
    # Wo^T with rows re-blocked: woT_blocks[m*8+d] = Wo^T rows of
    # (src core d, pair m) = global rows [(d*4+2m)*128 : +128]  (each core
    # owns 4 heads = 2 pairs of 2 heads; pair m rows = heads (4d+2m, +1))
    woT_full = Wo.T.astype(bf)                # [E_in(ctx rows), E_out]
    blocks = []
    for m in range(2):
        for d in range(NCORES):
            r0 = (d * 4 + 2 * m) * HD
            blocks.append(woT_full[r0:r0 + 2 * HD, :])
    woT = np.ascontiguousarray(np.concatenate(blocks, axis=0))

    in_maps = []
    for c in range(NCORES):
        qsl = slice(c * QH, (c + 1) * QH)
        ksl = slice(c * KVPC * HD, (c + 1) * KVPC * HD)
        wq = np.ascontiguousarray(Wq[qsl, :].T.astype(bf))
        wkv = np.ascontiguousarray(
            np.concatenate([Wk[ksl, :].T, Wv[ksl, :].T], axis=1).astype(bf))
        in_maps.append(dict(xT=xT, wqT=wq, wkvT=wkv, woT=woT,
                            cosT=cosT, sinT=sinT))
    return in_maps


def assemble_output(cfg, results):
    B, S, E = cfg["B"], cfg["S"], cfg["E"]
    NCORES = cfg["ncores"]
    TPB = S // NCORES // 2 * 2  # 256 tokens per (core, batch)
    out = np.empty((B, S, E), dtype=np.float32)
    for c in range(NCORES):
        o = np.asarray(results[c]["outT"]).astype(np.float32)
        for b in range(B):
            out[b, c * TPB:(c + 1) * TPB, :] = o[b * TPB:(b + 1) * TPB, :]
    return out


def kernel(x, mask, cos, sin, Wq, Wk, Wv, Wo):
    global LAST_RESULTS, _CACHED_NC
    _ensure_concourse()
    from concourse import bass_utils

    cfg = FULL_CFG
    if _CACHED_NC is None:
        _CACHED_NC = build_gqa(cfg)
    nc = _CACHED_NC
    in_maps = make_in_maps(cfg, x, cos, sin, Wq, Wk, Wv, Wo)
    res = bass_utils.run_bass_kernel_spmd(
        nc, in_maps, core_ids=list(range(cfg["ncores"])))
    LAST_RESULTS = res
    return assemble_output(cfg, res.results)
